# revision 12
# baseline (speedup 1.0000x reference)
"""Trainium2 Bass kernel for nn_MultiHeadAttention (B=2, S=2048, H=1024, 16 heads).

Sharding: 8 cores = 2 (batch) x 4 (head-groups of 4 heads). Each core computes
QKV projections for its 256-dim head slice, attention for its 4 heads, and a
partial output projection. Host sums the 4 head-group partials per batch and
adds the output bias.

On-chip layout: activations live transposed as [d, s] with the hidden/head dim
on partitions, so every matmul contraction runs on the PE partition axis with
no activation transposes (inputs are pre-transposed on the host during
sharding). Attention uses unnormalized exp scores with a fused ones-column in
V to produce row sums, normalizing the small [64, S] per-head output instead
of the [S, S] attention matrix.

Matmul operands are fp16 (1 cycle/row on the PE, 10-bit mantissa); all
accumulation is fp32 in PSUM. Measured end-to-end relative error ~8e-4.
"""

import sys

if "/opt/trn_rl_repo" not in sys.path:
    sys.path.insert(0, "/opt/trn_rl_repo")

import numpy as np

HIDDEN, HEADS, D_K, B, S = 1024, 16, 64, 2, 2048
G = 4              # head groups (tensor-parallel dim)
HPG = HEADS // G   # heads per group
DSL = HPG * D_K    # 256: d-slice per core
P = 128
QB = 512           # q-block size for attention tiling
N_QB = S // QB     # 4
KC = S // P        # 16 k-chunks
CC = HIDDEN // P   # 8 contraction chunks for projections
SCALE = 1.0 / np.sqrt(np.float32(D_K))


def _build_nc():
    import concourse.mybir as mybir
    import concourse.tile as tile
    from concourse.bacc import Bacc
    from concourse.masks import make_identity

    dt = mybir.dt
    f32 = dt.float32
    f16 = dt.float16

    nc = Bacc(None)

    # DRAM I/O (per-core shards, prepared on host; big operands as fp16)
    qT_d = nc.dram_tensor("qT", [HIDDEN, S], f16, kind="ExternalInput")
    kT_d = nc.dram_tensor("kT", [HIDDEN, S], f16, kind="ExternalInput")
    vT_d = nc.dram_tensor("vT", [HIDDEN, S], f16, kind="ExternalInput")
    wqT_d = nc.dram_tensor("wqT", [HIDDEN, DSL], f16, kind="ExternalInput")
    wkT_d = nc.dram_tensor("wkT", [HIDDEN, DSL], f16, kind="ExternalInput")
    wvT_d = nc.dram_tensor("wvT", [HIDDEN, DSL], f16, kind="ExternalInput")
    woT_d = nc.dram_tensor("woT", [DSL, HIDDEN], f16, kind="ExternalInput")
    bq_d = nc.dram_tensor("bq", [DSL], f32, kind="ExternalInput")
    bk_d = nc.dram_tensor("bk", [DSL], f32, kind="ExternalInput")
    bv_d = nc.dram_tensor("bv", [DSL], f32, kind="ExternalInput")
    y_d = nc.dram_tensor("y", [S, HIDDEN], f32, kind="ExternalOutput")

    with tile.TileContext(nc) as tc:
        with (
            tc.tile_pool(name="weights", bufs=1) as wpool,
            tc.tile_pool(name="qkvT", bufs=1) as qkvT_pool,
            tc.tile_pool(name="xT_out", bufs=1) as xT_pool,
            tc.tile_pool(name="small", bufs=1) as small,
        ):
            # ---- constants / weights ----
            ident = small.tile([P, P], f16)
            make_identity(nc, ident)
            ones_sb = small.tile([P, D_K], f32, tag="ones")
            nc.vector.memset(ones_sb[:], 1.0)

            w_sb = {}
            b_sb = {}
            for name, wd, bd in (
                ("q", wqT_d, bq_d),
                ("k", wkT_d, bk_d),
                ("v", wvT_d, bv_d),
            ):
                wt = wpool.tile([P, CC, DSL], f16, tag=f"w{name}")
                nc.sync.dma_start(wt[:], wd.rearrange("(c p) d -> p c d", p=P))
                w_sb[name] = wt
                bt = small.tile([P, DSL // P], f32, tag=f"b{name}")
                nc.sync.dma_start(bt[:], bd.rearrange("(o p) -> p o", p=P))
                b_sb[name] = bt
            woT_sb = wpool.tile([P, DSL // P, HIDDEN], f16, tag="wo")
            nc.sync.dma_start(woT_sb[:], woT_d.rearrange("(c p) e -> p c e", p=P))

            # ---- projections: XT = W^T @ xT + b, laid out [d, s] fp16 ----
            proj_out = {}
            for name in ("k", "q", "v"):
                proj_out[name] = qkvT_pool.tile(
                    [P, DSL // P, S], f16, tag=f"{name}T", name=f"{name}T"
                )

            with (
                tc.tile_pool(name="x_stream", bufs=2) as xpool,
                tc.tile_pool(name="proj_ps", bufs=2, space="PSUM") as proj_ps,
            ):
                for name, xd in (("k", kT_d), ("q", qT_d), ("v", vT_d)):
                    out_t = proj_out[name]
                    w_t = w_sb[name]
                    b_t = b_sb[name]
                    xt = xpool.tile([P, CC, S], f16, tag="x", name=f"x_{name}")
                    nc.sync.dma_start(
                        xt[:], xd.rearrange("(c p) s -> p c s", p=P)
                    )
                    for mc in range(DSL // P):
                        for ns in range(S // 512):
                            ps = proj_ps.tile([P, 512], f32, tag="proj")
                            for cc in range(CC):
                                nc.tensor.matmul(
                                    ps[:],
                                    w_t[:, cc, mc * P : (mc + 1) * P],
                                    xt[:, cc, ns * 512 : (ns + 1) * 512],
                                    start=(cc == 0),
                                    stop=(cc == CC - 1),
                                )
                            # copy + per-partition bias add, cast to fp16
                            nc.vector.tensor_scalar_add(
                                out_t[:, mc, ns * 512 : (ns + 1) * 512],
                                ps[:],
                                b_t[:, mc : mc + 1],
                            )

            QT, KT, VT = proj_out["q"], proj_out["k"], proj_out["v"]

            # ---- attention ----
            with (
                tc.tile_pool(name="expT", bufs=2) as exp_pool,
                tc.tile_pool(name="norm", bufs=2) as norm_pool,
                tc.tile_pool(name="sc_ps", bufs=3, space="PSUM") as sc_ps,
                tc.tile_pool(name="acc_ps", bufs=2, space="PSUM") as acc_ps,
                tc.tile_pool(name="vtr_ps", bufs=2, space="PSUM") as vtr_ps,
                tc.tile_pool(name="rb_ps", bufs=1, space="PSUM") as rb_ps_pool,
            ):
                # V' tiles: per head, [s, d] layout plus an extra ones column
                # (V'_h [128, KC, 65]); built by PE-transposing VT 64x128
                # blocks. The ones column turns attn@V into a fused
                # (unnormalized output, row sums) computation.
                vprime = []
                for h in range(HPG):
                    vp = xT_pool.tile([P, KC, D_K + 1], f16, tag=f"vp{h}")
                    nc.vector.memset(vp[:], 1.0)
                    hc, hp = divmod(h, 2)  # d-chunk, partition-half
                    pb = hp * D_K          # partition base 0 or 64
                    idn = ident[pb : pb + D_K, pb : pb + D_K]
                    for kc4 in range(KC // 4):
                        tp = vtr_ps.tile([P, 4, D_K], f16, tag="vtr")
                        for j in range(4):
                            kc = kc4 * 4 + j
                            nc.tensor.transpose(
                                tp[:, j, :],
                                VT[pb : pb + D_K, hc, kc * P : (kc + 1) * P],
                                idn,
                            )
                        nc.vector.tensor_copy(
                            vp[:, kc4 * 4 : kc4 * 4 + 4, 0:D_K], tp[:]
                        )
                    vprime.append(vp)

                # normalized attention outputs XT [256, 2048] = [128, 2, 2048]
                XT = xT_pool.tile([P, DSL // P, S], f16, tag="XT")

                for qb in range(N_QB):
                    qs = slice(qb * QB, (qb + 1) * QB)
                    for hpair in range(HPG // 2):
                        heads = (2 * hpair, 2 * hpair + 1)
                        expts = {}
                        accs = {}
                        for h in heads:
                            expts[h] = exp_pool.tile(
                                [P, KC, QB], f16, tag="exp", name=f"exp{h}"
                            )
                            accs[h] = acc_ps.tile(
                                [D_K + 1, QB], f32, tag="acc", name=f"acc{h}"
                            )
                        for kc in range(KC):
                            for h in heads:
                                hc, hp = divmod(h, 2)
                                pb = hp * D_K
                                # scoresT chunk [128k, QBq] = (K_h^T).T @ Q_h^T
                                ps = sc_ps.tile([P, QB], f32, tag="sc")
                                nc.tensor.matmul(
                                    ps[:],
                                    KT[pb : pb + D_K, hc, kc * P : (kc + 1) * P],
                                    QT[pb : pb + D_K, hc, qs],
                                    start=True,
                                    stop=True,
                                )
                                # exp(scores / sqrt(dk)) -> SBUF fp16
                                nc.scalar.activation(
                                    expts[h][:, kc, :],
                                    ps[:],
                                    mybir.ActivationFunctionType.Exp,
                                    scale=float(SCALE),
                                )
                                # X'T += (V'_h[kc]).T @ expT[kc]
                                nc.tensor.matmul(
                                    accs[h][:],
                                    vprime[h][:, kc, :],
                                    expts[h][:, kc, :],
                                    start=(kc == 0),
                                    stop=(kc == KC - 1),
                                )
                        for h in heads:
                            hc, hp = divmod(h, 2)
                            acc = accs[h]
                            # reciprocal of row sums (sums sit on partition 64
                            # of acc; stay lane-aligned through DVE), then
                            # broadcast across partitions via a K=1 fp32
                            # matmul against a ones column.
                            rec = norm_pool.tile([D_K + 1, QB], f32, tag="rec")
                            nc.vector.reciprocal(
                                rec[D_K : D_K + 1, :], acc[D_K : D_K + 1, :]
                            )
                            rb_ps = rb_ps_pool.tile(
                                [D_K, QB], f32, tag="rb", name=f"rb{h}"
                            )
                            nc.tensor.matmul(
                                rb_ps[:],
                                ones_sb[D_K : D_K + 1, :],
                                rec[D_K : D_K + 1, :],
                                start=True,
                                stop=True,
                            )
                            recb = norm_pool.tile([D_K, QB], f32, tag="recb")
                            nc.vector.tensor_copy(recb[:], rb_ps[:])
                            if hp == 0:
                                nc.vector.tensor_tensor(
                                    XT[0:D_K, hc, qs],
                                    acc[0:D_K, :],
                                    recb[:],
                                    mybir.AluOpType.mult,
                                )
                            else:
                                # result must land on partitions 64-127; DVE is
                                # lane-aligned, so normalize into a temp and
                                # shift partitions with an SBUF->SBUF DMA.
                                tmp = norm_pool.tile([D_K, QB], f16, tag="xtmp")
                                nc.vector.tensor_tensor(
                                    tmp[:],
                                    acc[0:D_K, :],
                                    recb[:],
                                    mybir.AluOpType.mult,
                                )
                                nc.sync.dma_start(XT[D_K:P, hc, qs], tmp[:])

            # ---- output projection: y[s, e] = XT.T @ woT ----
            with (
                tc.tile_pool(name="y_out", bufs=1) as ypool,
                tc.tile_pool(name="y_ps", bufs=2, space="PSUM") as y_ps,
            ):
                y_sb = ypool.tile([P, S // P, HIDDEN], f32)
                for sc in range(S // P):
                    for ec in range(HIDDEN // 512):
                        ps = y_ps.tile([P, 512], f32, tag="yps")
                        for dc in range(DSL // P):
                            nc.tensor.matmul(
                                ps[:],
                                XT[:, dc, sc * P : (sc + 1) * P],
                                woT_sb[:, dc, ec * 512 : (ec + 1) * 512],
                                start=(dc == 0),
                                stop=(dc == DSL // P - 1),
                            )
                        nc.scalar.copy(
                            y_sb[:, sc, ec * 512 : (ec + 1) * 512], ps[:]
                        )
                nc.sync.dma_start(
                    y_d.rearrange("(sc p) e -> p sc e", p=P), y_sb[:]
                )

    nc.finalize()
    return nc


_NC_CACHE = None


def _get_nc():
    global _NC_CACHE
    if _NC_CACHE is None:
        _NC_CACHE = _build_nc()
    return _NC_CACHE


def make_in_maps(q, k, v, Wq, bq, Wk, bk, Wv, bv, Wo):
    """Host-side sharding: per-core input dicts (core = b * G + g)."""
    f16 = np.float16
    qT = [np.ascontiguousarray(q[b].T).astype(f16) for b in range(B)]
    kT = [np.ascontiguousarray(k[b].T).astype(f16) for b in range(B)]
    vT = [np.ascontiguousarray(v[b].T).astype(f16) for b in range(B)]
    in_maps = []
    for core in range(B * G):
        b, g = divmod(core, G)
        sl = slice(g * DSL, (g + 1) * DSL)
        in_maps.append(
            {
                "qT": qT[b],
                "kT": kT[b],
                "vT": vT[b],
                "wqT": np.ascontiguousarray(Wq[sl, :].T).astype(f16),
                "wkT": np.ascontiguousarray(Wk[sl, :].T).astype(f16),
                "wvT": np.ascontiguousarray(Wv[sl, :].T).astype(f16),
                "woT": np.ascontiguousarray(Wo[:, sl].T).astype(f16),
                "bq": np.ascontiguousarray(bq[sl], np.float32),
                "bk": np.ascontiguousarray(bk[sl], np.float32),
                "bv": np.ascontiguousarray(bv[sl], np.float32),
            }
        )
    return in_maps


def kernel(q, k, v, Wq, bq, Wk, bk, Wv, bv, Wo, bo):
    from concourse.bass_utils import run_bass_kernel_spmd

    q, k, v = (np.asarray(a, np.float32) for a in (q, k, v))
    Wq, Wk, Wv, Wo = (np.asarray(a, np.float32) for a in (Wq, Wk, Wv, Wo))
    bq, bk, bv, bo = (np.asarray(a, np.float32) for a in (bq, bk, bv, bo))

    nc = _get_nc()
    in_maps = make_in_maps(q, k, v, Wq, bq, Wk, bk, Wv, bv, Wo)
    res = run_bass_kernel_spmd(nc, in_maps, core_ids=list(range(B * G)))

    out = np.zeros((B, S, HIDDEN), np.float32)
    for b in range(B):
        acc = np.zeros((S, HIDDEN), np.float32)
        for g in range(G):
            acc += res.results[b * G + g]["y"]
        out[b] = acc + bo
    return out
